# revision 5
# baseline (speedup 1.0000x reference)
"""Trainium2 Bass kernel for nn_Attention_1537598292670.

reference:
    scores  = einsum('bqh,bkh->bqk', ys, hs)      # B=16, TQ=TK=2048, H=512
    weights = softmax(scores, axis=-1)
    out     = einsum('bqk,bkh->bqh', weights, hs)

Sharding: data-parallel over batch — 16 batches across 8 NeuronCores,
2 batches per core, no collectives.

Per-core kernel, per batch:
  - load hs natural [k,h]; build hsT [h,k] via PE transposes (f32) +
    DVE copies that round to f32r
  - per 128-row q-tile: ysT via PE transposes; scores = ysT.T @ hsT as
    f32r matmuls (4x the f32 rate); softmax with exact row max (DVE
    reduce_max negate, ACT fused exp+row-sum, DVE reciprocal); exp
    weights transposed on PE; ctx = wT.T @ hs_natr (f32r); rows scaled
    by 1/sum; DMA out.

Notes for this toolchain:
  - walrus here accepts only ONE semaphore wait per instruction; extra
    waits are split onto injected no-ops after Tile scheduling.
  - f32r operands must be produced "rounded": DVE copies with f32r
    output dtype, or DMA from f32r-declared DRAM.
  - PE transposes run in f32 mode; the f32r weight-load path is only
    used by the fused matmul (the f32r transpose/ldweights path
    misbehaves on hardware).
"""
import numpy as np

B, TQ, TK, H = 16, 2048, 2048, 512
N_CORES = 8
B_LOC = B // N_CORES           # 2 batches per core
NQT = TQ // 128                # 16 q-tiles per batch
NKT = TK // 128                # 16 k-tiles (128 rows each)
NKC = TK // 512                # 4 score chunks of 512
NHJ = H // 128                 # 4 h-blocks

_CACHE = {}


def _split_waits(nc, max_waits=1):
    import bass_rust
    import concourse.mybir as mybir

    ctr = 0
    for f in nc.m.functions:
        for blk in f.blocks:
            new = []
            for inst in blk.instructions:
                si = inst.sync_info
                if si is not None and len(si.on_wait) > max_waits:
                    waits = list(si.on_wait)
                    extra, keep = waits[:-max_waits], waits[-max_waits:]
                    for w in extra:
                        ctr += 1
                        nop = mybir.InstNoOp(
                            name=f"I-waitnop-{ctr}",
                            bass_nofuse=True,
                            text_hint="waitsplit",
                        )
                        nop.engine = inst.engine
                        nop.sync_info = bass_rust.SyncInfo(on_wait=[w], on_update=[])
                        new.append(nop)
                    inst.sync_info = bass_rust.SyncInfo(
                        on_wait=keep, on_update=list(si.on_update)
                    )
                new.append(inst)
            blk.instructions = new
    return ctr


def _build(split=True):
    import concourse.bass as bass
    import concourse.mybir as mybir
    import concourse.tile as tile
    from concourse.masks import make_identity

    F32 = mybir.dt.float32
    F32R = mybir.dt.float32r
    AX = mybir.AxisListType
    AF = mybir.ActivationFunctionType

    nc = bass.Bass()
    ys = nc.declare_dram_parameter("ys", [B_LOC, TQ, H], F32, isOutput=False)
    hs = nc.declare_dram_parameter("hs", [B_LOC, TK, H], F32, isOutput=False)
    hsr = nc.declare_dram_parameter("hsr", [B_LOC, TK, H], F32R, isOutput=False)
    out = nc.declare_dram_parameter("out", [B_LOC, TQ, H], F32, isOutput=True)

    with tile.TileContext(nc) as tc:
        with (
            tc.tile_pool(name="const", bufs=1) as const,
            tc.tile_pool(name="hsp", bufs=1) as hsp,
            tc.tile_pool(name="qt", bufs=2) as qt,
            tc.tile_pool(name="stats", bufs=8) as stats,
            tc.tile_pool(name="ps4", bufs=4, space="PSUM") as ps4,
            tc.tile_pool(name="ps2t", bufs=2, space="PSUM") as ps2t,
            tc.tile_pool(name="ps2o", bufs=2, space="PSUM") as ps2o,
        ):
            ident32 = const.tile([128, 128], F32)
            make_identity(nc, ident32)

            for b in range(B_LOC):
                # ---- per-batch hs structures ----
                hs_nat = hsp.tile([128, NKT, H], F32, tag="hs_nat")    # [k_p, t, h]
                hs_natr = hsp.tile([128, NKT, H], F32R, tag="hs_natr")
                for t in range(NKT):
                    nc.sync.dma_start(
                        out=hs_nat[:, t, :], in_=hs[b, t * 128:(t + 1) * 128, :]
                    )
                    nc.sync.dma_start(
                        out=hs_natr[:, t, :], in_=hsr[b, t * 128:(t + 1) * 128, :]
                    )
                # hsT[p, j, k] = hs[b, k, j*128+p]
                hsT = hsp.tile([128, NHJ, TK], F32R, tag="hsT")
                for j in range(NHJ):
                    for tg in range(NKT // 4):
                        ps_t = ps2t.tile([128, 4, 128], F32, tag="ps_t")
                        for u in range(4):
                            t = tg * 4 + u
                            nc.tensor.transpose(
                                ps_t[:, u, :],
                                hs_nat[:, t, j * 128:(j + 1) * 128],
                                ident32,
                            )
                        nc.vector.tensor_copy(
                            hsT[:, j, tg * 512:(tg + 1) * 512],
                            ps_t.rearrange("p a b -> p (a b)"),
                        )

                # ---- q-tiles ----
                for i in range(NQT):
                    ys_nat = qt.tile([128, H], F32, tag="ys_nat")
                    nc.sync.dma_start(
                        out=ys_nat, in_=ys[b, i * 128:(i + 1) * 128, :]
                    )
                    ysT = qt.tile([128, NHJ, 128], F32R, tag="ysT")
                    ps_y = ps2t.tile([128, 4, 128], F32, tag="ps_t")
                    for j in range(NHJ):
                        nc.tensor.transpose(
                            ps_y[:, j, :], ys_nat[:, j * 128:(j + 1) * 128], ident32
                        )
                    nc.vector.tensor_copy(
                        ysT.rearrange("p a b -> p (a b)"),
                        ps_y.rearrange("p a b -> p (a b)"),
                    )

                    # scores in 4 chunks of 512 k
                    ps_s = [None] * NKC
                    pmax = stats.tile([128, NKC], F32, tag="pmax")
                    for c in range(NKC):
                        ps_s[c] = ps4.tile([128, 512], F32, tag="ps_s", name="ps_s")
                        for j in range(NHJ):
                            nc.tensor.matmul(
                                ps_s[c],
                                ysT[:, j, :],
                                hsT[:, j, c * 512:(c + 1) * 512],
                                start=(j == 0),
                                stop=(j == NHJ - 1),
                            )
                        nc.vector.reduce_max(pmax[:, c:c + 1], ps_s[c], axis=AX.X)

                    nmax = stats.tile([128, 1], F32, tag="nmax")
                    nc.vector.reduce_max(nmax, pmax, axis=AX.X, negate=True)

                    p_sb = qt.tile([128, TK], F32, tag="p_sb")
                    sums4 = stats.tile([128, NKC], F32, tag="sums4")
                    for c in range(NKC):
                        nc.scalar.activation(
                            out=p_sb[:, c * 512:(c + 1) * 512],
                            in_=ps_s[c],
                            func=AF.Exp,
                            bias=nmax,
                            scale=1.0,
                            accum_out=sums4[:, c:c + 1],
                        )
                    ssum = stats.tile([128, 1], F32, tag="ssum")
                    nc.vector.reduce_sum(ssum, sums4, axis=AX.X)
                    recip = stats.tile([128, 1], F32, tag="recip")
                    nc.vector.reciprocal(recip, ssum)

                    # wT[p, t, q] = p_sb[q, t*128+p]
                    wT = qt.tile([128, NKT, 128], F32R, tag="wT")
                    for tg in range(NKT // 4):
                        ps_w = ps2t.tile([128, 4, 128], F32, tag="ps_t", name="ps_w")
                        for u in range(4):
                            t = tg * 4 + u
                            nc.tensor.transpose(
                                ps_w[:, u, :],
                                p_sb[:, t * 128:(t + 1) * 128],
                                ident32,
                            )
                        nc.vector.tensor_copy(
                            wT[:, tg * 4:(tg + 1) * 4, :].rearrange(
                                "p a b -> p (a b)"
                            ),
                            ps_w.rearrange("p a b -> p (a b)"),
                        )

                    # ctx = wT.T @ hs_natr  -> [q, H]
                    ps_o = ps2o.tile([128, H], F32, tag="ps_o")
                    for t in range(NKT):
                        nc.tensor.matmul(
                            ps_o,
                            wT[:, t, :],
                            hs_natr[:, t, :],
                            start=(t == 0),
                            stop=(t == NKT - 1),
                        )
                    o_sb = qt.tile([128, H], F32, tag="o_sb")
                    nc.vector.tensor_scalar_mul(o_sb, ps_o, recip)
                    nc.sync.dma_start(
                        out=out[b, i * 128:(i + 1) * 128, :], in_=o_sb
                    )
    if split:
        _split_waits(nc)
    return nc


def kernel(ys: np.ndarray, hs: np.ndarray) -> np.ndarray:
    from concourse.bass_utils import run_bass_kernel_spmd

    if "nc" not in _CACHE:
        _CACHE["nc"] = _build()
    nc = _CACHE["nc"]

    ys = np.ascontiguousarray(np.asarray(ys, dtype=np.float32))
    hs = np.ascontiguousarray(np.asarray(hs, dtype=np.float32))
    in_maps = [
        {
            "ys": ys[c * B_LOC:(c + 1) * B_LOC],
            "hs": hs[c * B_LOC:(c + 1) * B_LOC],
            "hsr": hs[c * B_LOC:(c + 1) * B_LOC],
        }
        for c in range(N_CORES)
    ]
    res = run_bass_kernel_spmd(nc, in_maps, list(range(N_CORES)))
    return np.concatenate([res.results[c]["out"] for c in range(N_CORES)], axis=0)


# revision 19
# speedup vs baseline: 20859.2372x; 20859.2372x over previous
"""Trainium2 Bass kernel for nn_Attention_1537598292670.

reference:
    scores  = einsum('bqh,bkh->bqk', ys, hs)      # B=16, TQ=TK=2048, H=512
    weights = softmax(scores, axis=-1)
    out     = einsum('bqk,bkh->bqh', weights, hs)

Sharding: data-parallel over batch — 16 batches across 8 NeuronCores,
2 batches per core, no collectives.

Per-core kernel, per batch:
  - load hs [k,h] (f32r); build hsT [h,k] via PE transposes (f32-mode on
    bitcast views) + DVE copies that round to f32r
  - per 128-row q-tile: ysT via PE transposes; scores = ysT.T @ hsT as
    f32r matmuls (4x the f32 rate). Softmax is two-half flash style:
    each 1024-wide half exps against its own row max right after its
    matmuls finish (no global-max barrier, so the PE never stalls on
    softmax latency); halves are rescaled and combined after their
    separate AV matmuls.

Toolchain notes:
  - this walrus accepts only ONE semaphore wait per instruction; extra
    waits are split onto injected no-ops after Tile scheduling.
  - f32r operands must be produced "rounded": DVE copies with f32r
    output dtype, or DMA from f32r-declared DRAM.
  - PE transposes run in f32 mode (the f32r transpose path hangs on
    hardware); f32r inputs are bitcast to f32 for transposing.
"""
import numpy as np

B, TQ, TK, H = 16, 2048, 2048, 512
N_CORES = 8
B_LOC = B // N_CORES           # 2 batches per core
NQT = TQ // 128                # 16 q-tiles per batch
NKT = TK // 128                # 16 k-tiles (128 rows each)
NHJ = H // 128                 # 4 h-blocks
KHALF = TK // 2                # 1024 k per softmax half

_CACHE = {}


def _split_waits(nc, max_waits=1):
    import bass_rust
    import concourse.mybir as mybir

    ctr = 0
    for f in nc.m.functions:
        for blk in f.blocks:
            new = []
            for inst in blk.instructions:
                si = inst.sync_info
                if si is not None and len(si.on_wait) > max_waits:
                    waits = list(si.on_wait)
                    extra, keep = waits[:-max_waits], waits[-max_waits:]
                    for w in extra:
                        ctr += 1
                        nop = mybir.InstNoOp(
                            name=f"I-waitnop-{ctr}",
                            bass_nofuse=True,
                            text_hint="waitsplit",
                        )
                        nop.engine = inst.engine
                        nop.sync_info = bass_rust.SyncInfo(on_wait=[w], on_update=[])
                        new.append(nop)
                    inst.sync_info = bass_rust.SyncInfo(
                        on_wait=keep, on_update=list(si.on_update)
                    )
                new.append(inst)
            blk.instructions = new
    return ctr


def _build(split=True):
    import concourse.bass as bass
    import concourse.mybir as mybir
    import concourse.tile as tile
    from concourse.masks import make_identity

    F32 = mybir.dt.float32
    F32R = mybir.dt.float32r
    AX = mybir.AxisListType
    AF = mybir.ActivationFunctionType
    ALU = mybir.AluOpType

    nc = bass.Bass()
    ys = nc.declare_dram_parameter("ys", [B_LOC, TQ, H], F32R, isOutput=False)
    hs = nc.declare_dram_parameter("hs", [B_LOC, TK, H], F32R, isOutput=False)
    out = nc.declare_dram_parameter("out", [B_LOC, TQ, H], F32, isOutput=True)

    with tile.TileContext(nc) as tc:
        with (
            tc.tile_pool(name="const", bufs=1) as const,
            tc.tile_pool(name="hsp", bufs=2) as hsp,
            tc.tile_pool(name="qt", bufs=2) as qt,
            tc.tile_pool(name="stats", bufs=8) as stats,
            tc.tile_pool(name="ps_s", bufs=2, space="PSUM") as psum_s,
            tc.tile_pool(name="ps_t", bufs=2, space="PSUM") as psum_t,
            tc.tile_pool(name="ps_o", bufs=2, space="PSUM") as psum_o,
        ):
            ident32 = const.tile([128, 128], F32)
            make_identity(nc, ident32)

            for b in range(B_LOC):
                # ---- per-batch hs structures ----
                hs_nat = hsp.tile([128, NKT, H], F32R, tag="hs_nat")   # [k_p, t, h]
                for t in range(NKT):
                    nc.sync.dma_start(
                        out=hs_nat[:, t, :], in_=hs[b, t * 128:(t + 1) * 128, :]
                    )
                # hsT[p, j, k] = hs[b, k, j*128+p]
                hsT = hsp.tile([128, NHJ, TK], F32R, tag="hsT")
                for j in range(NHJ):
                    for tg in range(NKT // 4):
                        ps_t = psum_t.tile([128, 4, 128], F32, tag="ps_t")
                        for u in range(4):
                            t = tg * 4 + u
                            nc.tensor.transpose(
                                ps_t[:, u, :],
                                hs_nat[:, t, j * 128:(j + 1) * 128].bitcast(F32),
                                ident32,
                            )
                        nc.vector.tensor_copy(
                            hsT[:, j, tg * 512:(tg + 1) * 512],
                            ps_t.rearrange("p a b -> p (a b)"),
                        )

                # ---- q-tiles ----
                for i in range(NQT):
                    ys_nat = qt.tile([128, H], F32R, tag="ys_nat")
                    nc.sync.dma_start(
                        out=ys_nat, in_=ys[b, i * 128:(i + 1) * 128, :]
                    )
                    ysT = qt.tile([128, NHJ, 128], F32R, tag="ysT")
                    ps_y = psum_t.tile([128, 4, 128], F32, tag="ps_t")
                    for j in range(NHJ):
                        nc.tensor.transpose(
                            ps_y[:, j, :],
                            ys_nat[:, j * 128:(j + 1) * 128].bitcast(F32),
                            ident32,
                        )
                    nc.vector.tensor_copy(
                        ysT.rearrange("p a b -> p (a b)"),
                        ps_y.rearrange("p a b -> p (a b)"),
                    )

                    # two-half flash softmax over k
                    p_sb = qt.tile([128, TK], F32, tag="p_sb")
                    wT = qt.tile([128, NKT, 128], F32R, tag="wT")
                    nmh = stats.tile([128, 2], F32, tag="nmh")     # -max per half
                    sums2 = stats.tile([128, 2], F32, tag="sums2")
                    ps_oh = []
                    for hn in range(2):
                        ph = psum_s.tile([128, 2, 512], F32, tag="ps_s", name="ps_s")
                        for cc in range(2):
                            c = hn * 2 + cc
                            for j in range(NHJ):
                                nc.tensor.matmul(
                                    ph[:, cc, :],
                                    ysT[:, j, :],
                                    hsT[:, j, c * 512:(c + 1) * 512],
                                    start=(j == 0),
                                    stop=(j == NHJ - 1),
                                )
                        nc.vector.reduce_max(
                            nmh[:, hn:hn + 1],
                            ph.rearrange("p a b -> p (a b)"),
                            axis=AX.X,
                            negate=True,
                        )
                        # exp(s - max_h) for this half, with row-sum accum
                        nc.scalar.activation(
                            out=p_sb[:, hn * KHALF:(hn + 1) * KHALF],
                            in_=ph.rearrange("p a b -> p (a b)"),
                            func=AF.Exp,
                            bias=nmh[:, hn:hn + 1],
                            scale=1.0,
                            accum_out=sums2[:, hn:hn + 1],
                        )
                        # wT for this half
                        for tg in range(2):
                            ps_w = psum_t.tile([128, 4, 128], F32, tag="ps_t",
                                               name="ps_w")
                            for u in range(4):
                                t = hn * 8 + tg * 4 + u
                                nc.tensor.transpose(
                                    ps_w[:, u, :],
                                    p_sb[:, t * 128:(t + 1) * 128],
                                    ident32,
                                )
                            nc.vector.tensor_copy(
                                wT[:, hn * 8 + tg * 4:hn * 8 + (tg + 1) * 4, :]
                                .rearrange("p a b -> p (a b)"),
                                ps_w.rearrange("p a b -> p (a b)"),
                            )
                        # AV for this half (two sub-groups, gated per wT quarter)
                        ph_o = psum_o.tile([128, H], F32, tag="ps_o", name="ps_o")
                        ps_oh.append(ph_o)
                        for u in range(8):
                            t = hn * 8 + u
                            nc.tensor.matmul(
                                ph_o,
                                wT[:, t, :],
                                hs_nat[:, t, :],
                                start=(u == 0),
                                stop=(u == 7),
                            )

                    # combine halves: m = max(mA, mB); f_h = exp(m_h - m)
                    # nmh holds -m_h, so -m = min over nmh and f_h = exp(nm - nmh)
                    nm = stats.tile([128, 1], F32, tag="nm")
                    nc.vector.tensor_reduce(nm, nmh, axis=AX.X, op=ALU.min)
                    d2 = stats.tile([128, 2], F32, tag="d2")
                    nc.vector.tensor_scalar(
                        d2, nmh, -1.0, nm, op0=ALU.mult, op1=ALU.add
                    )
                    f2 = stats.tile([128, 2], F32, tag="f2")
                    nc.scalar.activation(f2, d2, AF.Exp, bias=0.0, scale=1.0)
                    # total sum = sum_h S_h * f_h ; g_h = f_h / total
                    sf2 = stats.tile([128, 2], F32, tag="sf2")
                    nc.vector.tensor_tensor(out=sf2, in0=sums2, in1=f2, op=ALU.mult)
                    ssum = stats.tile([128, 1], F32, tag="ssum")
                    nc.vector.reduce_sum(ssum, sf2, axis=AX.X)
                    recip = stats.tile([128, 1], F32, tag="recip")
                    nc.vector.reciprocal(recip, ssum)
                    g2 = stats.tile([128, 2], F32, tag="g2")
                    nc.vector.tensor_scalar_mul(g2, f2, recip)

                    o_half = qt.tile([128, 2, H], F32, tag="o_half")
                    for hn in range(2):
                        nc.scalar.activation(
                            out=o_half[:, hn, :],
                            in_=ps_oh[hn],
                            func=AF.Identity,
                            bias=0.0,
                            scale=g2[:, hn:hn + 1],
                        )
                    o_sb = qt.tile([128, H], F32, tag="o_sb")
                    nc.vector.tensor_tensor(
                        out=o_sb, in0=o_half[:, 0, :], in1=o_half[:, 1, :],
                        op=ALU.add,
                    )
                    nc.sync.dma_start(
                        out=out[b, i * 128:(i + 1) * 128, :], in_=o_sb
                    )
    if split:
        _split_waits(nc)
    return nc


def kernel(ys: np.ndarray, hs: np.ndarray) -> np.ndarray:
    from concourse.bass_utils import run_bass_kernel_spmd

    if "nc" not in _CACHE:
        _CACHE["nc"] = _build()
    nc = _CACHE["nc"]

    ys = np.ascontiguousarray(np.asarray(ys, dtype=np.float32))
    hs = np.ascontiguousarray(np.asarray(hs, dtype=np.float32))
    in_maps = [
        {
            "ys": ys[c * B_LOC:(c + 1) * B_LOC],
            "hs": hs[c * B_LOC:(c + 1) * B_LOC],
        }
        for c in range(N_CORES)
    ]
    res = run_bass_kernel_spmd(nc, in_maps, list(range(N_CORES)))
    return np.concatenate([res.results[c]["out"] for c in range(N_CORES)], axis=0)
